# revision 19
# baseline (speedup 1.0000x reference)
"""v4: batch-major Trainium kernel for the coupled-pendulum ODE.

Math (verified on host): classical 3-stage Runge-Kutta-Nystrom order 4,
integrating  theta'' = -omega0^2 sin(theta) + coupling*(ring laplacian)
in rescaled units: tau = omega0*t, angles in TURNS (theta = theta_phys/2pi):
    d2theta/dtau2 = cp*(lap theta) - sin(2pi theta)/(2pi),  cp = coupling/omega0^2.
Working variable kt = theta''/cp = (nsum - 2 theta) + c*ns with
ns = -sin(2pi theta), c = 1/(2pi cp), nsum = theta[n-1] + theta[n+1].

Layout C: per core a [128, 512] tile — 128 partitions = batch rows,
512 free = ring positions. The ring Laplacian is then free-dim shifted
adds (scalar_tensor_tensor, which runs in the DVE 2x perf mode for fp32
SBUF operands) instead of PE matmuls; ring wraparound is handled by two
[128,1] edge fixup ops per stencil. No PE, no PSUM, no host transposes.

Engines: DVE custom TURNS_WRAP (exact frac via the 1.5*2^23 magic trick;
ACT Sin is only valid on ~[-pi,pi]) + most stt ops; ACT the three Sin
activations; Pool (gpsimd) the off-critical-path stt ops.
"""

import math

import numpy as np

import concourse.bacc as bacc
import concourse.bass as bass
import concourse.dve_ops as dve_ops
import concourse.mybir as mybir
import concourse.tile as tile
from concourse.bass_utils import run_bass_kernel_spmd
from concourse.dve_spec import C0, C1, C2, Spec, Src0, Src1, _has_src1, lower
from concourse.dve_uop import DveOpSpec

F32 = mybir.dt.float32
AF = mybir.ActivationFunctionType
OP = mybir.AluOpType

N_CORES = 8
B, N = 1024, 512
PB = B // N_CORES            # 128 batch rows per core

NSTEPS = 96
T_END = 2.0
TWO_PI = 2 * math.pi
MAGIC = 12582912.0           # 1.5 * 2**23: fp32 round-to-int trick


def _register_custom_op(name, body, reference):
    for op in dve_ops.OPS:
        if op.name == name:
            return op
    idx = dve_ops._CUSTOM_DVE_ROW_BASE + len(dve_ops.OPS)
    assert idx < 0x20
    spec = Spec(body=body, reference=reference)
    shas = {}
    for ver in ("v3", "v4"):
        try:
            uops = lower(spec, ver=ver)
            tmp = DveOpSpec(name=name, opcode=idx, uops=uops,
                            rd1_en=_has_src1(spec))
            shas[ver] = tmp.sha(ver)
        except Exception:
            pass
    op = dve_ops.DveOp(name, spec, subdim=False, uops_sha=shas)
    dve_ops.OPS.append(op)
    dve_ops._SUB_OPCODE_FOR_NAME[name] = idx
    dve_ops.CUSTOM_DVE_SPECS[name] = spec
    return op


def _f32(v):
    return np.float32(v)


_tw_z = Src0 * C0 + Src1 * C1
TURNS_WRAP = _register_custom_op(
    "TURNS_WRAP_ANT",
    _tw_z - ((_tw_z + C2) - C2),
    lambda in0, in1, s0, s1, imm2: (
        lambda z: z - ((z + _f32(imm2)) - _f32(imm2)))(
        (in0.astype(np.float32) * _f32(s0)
         + in1.astype(np.float32) * _f32(s1)).astype(np.float32)),
)


def _build(nsteps: int, omega0: float, coupling: float) -> bass.Bass:
    tau_end = omega0 * T_END
    h = tau_end / nsteps
    cp = coupling / (omega0 * omega0)
    c_sin = 1.0 / (TWO_PI * cp)     # ns coefficient inside kt
    s2 = cp * h * h / 8.0           # p2 = th + (h/2) u + s2*kt1
    s3 = cp * h * h / 2.0           # p3 = t  + s3*kt2
    sg = cp * h * h / 3.0           # th' = t + sg*(kt1/2 + kt2)
    su = cp * h / 6.0               # u'  = u + su*(kt1 + 4 kt2 + kt3)

    mu = sg / 2.0                   # q = t + mu*kt1
    kp = cp * h * h / 12.0          # HU' = HU + kp*(kt1 + 4 kt2 + kt3)

    nc = bacc.Bacc("TRN2", target_bir_lowering=False, debug=False,
                   num_devices=N_CORES)
    x_in = nc.dram_tensor("x", [PB, N], F32, kind="ExternalInput")
    out = nc.dram_tensor("out", [PB, N], F32, kind="ExternalOutput")

    with tile.TileContext(nc) as tc:
        with (
            tc.tile_pool(name="state", bufs=1) as state,
            tc.tile_pool(name="tmp", bufs=2) as tmp,
        ):
            th = state.tile([PB, N], F32, name="th", tag="th")
            th2 = state.tile([PB, N], F32, name="th2", tag="th2")
            hu = state.tile([PB, N], F32, name="hu", tag="hu")
            hu2 = state.tile([PB, N], F32, name="hu2", tag="hu2")

            xs = tmp.tile([PB, N], F32, name="xs", tag="xs")
            nc.gpsimd.dma_start(xs[:], x_in[:])
            # theta0 = x - 0.5 (turns); HU = (h/2) u = 0
            nc.scalar.activation(th[:], xs[:], AF.Copy, bias=-0.5, scale=1.0)
            nc.vector.memset(hu[:], 0.0)

            ths = [th, th2]
            hus = [hu, hu2]
            prev_kq = [None]

            H = N // 2
            HV = [(0, H), (H, N)]

            def lap_of(p, nm):
                """lap[n] = p[n-1] - 2 p[n] + p[n+1] on the ring, as
                scalar-free Pool subtracts (d1 = shifted difference, lap =
                d1 - d1[-1]) in column halves (so dependents pipeline at
                half-tile granularity), plus [128,1] DVE edge fixups."""
                d1 = tmp.tile([PB, N], F32, name="d" + nm, tag="d" + nm)
                lp = tmp.tile([PB, N], F32, name="l" + nm, tag="l" + nm)
                # d1[j] = p[j+1] - p[j]; edge j=511 wraps to p[0].
                # Strict A/B halves (A ops read only A columns of the
                # producer, B ops only B columns) with the crossing columns
                # handled by [128,1] DVE seam ops, so the A-half of every
                # stage pipelines without waiting for the B-half upstream.
                nc.gpsimd.tensor_tensor(d1[:, 0:H - 1], p[:, 1:H],
                                        p[:, 0:H - 1], OP.subtract)
                nc.vector.tensor_tensor(d1[:, H - 1:H], p[:, H:H + 1],
                                        p[:, H - 1:H], OP.subtract)
                nc.gpsimd.tensor_tensor(d1[:, H:N - 1], p[:, H + 1:N],
                                        p[:, H:N - 1], OP.subtract)
                nc.vector.tensor_tensor(d1[:, N - 1:N], p[:, 0:1],
                                        p[:, N - 1:N], OP.subtract)
                # lap[j] = d1[j] - d1[j-1]; edge j=0 wraps to d1[511]
                nc.gpsimd.tensor_tensor(lp[:, 1:H - 1], d1[:, 1:H - 1],
                                        d1[:, 0:H - 2], OP.subtract)
                nc.vector.tensor_tensor(lp[:, 0:1], d1[:, 0:1],
                                        d1[:, N - 1:N], OP.subtract)
                nc.vector.tensor_tensor(lp[:, H - 1:H], d1[:, H - 1:H],
                                        d1[:, H - 2:H - 1], OP.subtract)
                nc.gpsimd.tensor_tensor(lp[:, H:N], d1[:, H:N],
                                        d1[:, H - 1:N - 1], OP.subtract)
                return lp

            def step(i):
                th, th_new = ths
                hu, hu_new = hus
                t_ = tmp.tile([PB, N], F32, name="t_", tag="t_")
                v2 = tmp.tile([PB, N], F32, name="v2", tag="v2")
                q = tmp.tile([PB, N], F32, name="q", tag="q")
                w2 = tmp.tile([PB, N], F32, name="w2", tag="w2")
                w3 = tmp.tile([PB, N], F32, name="w3", tag="w3")
                sn2 = tmp.tile([PB, N], F32, name="sn2", tag="sn2")
                sn3 = tmp.tile([PB, N], F32, name="sn3", tag="sn3")
                k1 = tmp.tile([PB, N], F32, name="k1", tag="k1")
                k2 = tmp.tile([PB, N], F32, name="k2", tag="k2")
                k3 = tmp.tile([PB, N], F32, name="k3", tag="k3")
                sk1 = tmp.tile([PB, N], F32, name="sk1", tag="sk1")
                cs3 = tmp.tile([PB, N], F32, name="cs3", tag="cs3")
                mm1 = tmp.tile([PB, N], F32, name="mm1", tag="mm1")
                sm = tmp.tile([PB, N], F32, name="sm", tag="sm")
                p2 = tmp.tile([PB, N], F32, name="p2", tag="p2")
                p3 = tmp.tile([PB, N], F32, name="p3", tag="p3")
                m = tmp.tile([PB, N], F32, name="m", tag="m")
                m2 = tmp.tile([PB, N], F32, name="m2", tag="m2")

                def halves(fn):
                    for lo, hi in HV:
                        fn(lo, hi)

                # ---- eval 1 at theta: wrap+sin precomputed by step i-1 ----
                w1 = tmp.tile([PB, N], F32, name="w1", tag="w1")
                sn1 = tmp.tile([PB, N], F32, name="sn1", tag="sn1")
                if prev_kq[0] is None:
                    halves(lambda lo, hi: nc.vector._custom_dve(
                        TURNS_WRAP, out=w1[:, lo:hi], in0=th[:, lo:hi],
                        in1=th[:, lo:hi], s0=1.0, s1=0.0, imm2=MAGIC))
                else:
                    pk, pq = prev_kq[0]
                    halves(lambda lo, hi: nc.vector._custom_dve(
                        TURNS_WRAP, out=w1[:, lo:hi], in0=pk[:, lo:hi],
                        in1=pq[:, lo:hi], s0=sg, s1=1.0, imm2=MAGIC))
                halves(lambda lo, hi: nc.scalar.activation(
                    sn1[:, lo:hi], w1[:, lo:hi], AF.Sin, scale=-TWO_PI))
                lp1 = lap_of(th, "p1")
                halves(lambda lo, hi: nc.gpsimd.tensor_tensor(
                    v2[:, lo:hi], th[:, lo:hi], hu[:, lo:hi], OP.add))
                halves(lambda lo, hi: nc.gpsimd.tensor_tensor(
                    t_[:, lo:hi], v2[:, lo:hi], hu[:, lo:hi], OP.add))
                halves(lambda lo, hi: nc.vector.scalar_tensor_tensor(
                    k1[:, lo:hi], sn1[:, lo:hi], c_sin, lp1[:, lo:hi],
                    OP.mult, OP.add))

                # ---- eval 2 at p2 = v2 + s2 kt1 (wrap first: sin is the
                # long pole; p2 fills the DVE pipe while ACT runs) ----
                halves(lambda lo, hi: nc.vector.scalar_tensor_tensor(
                    p2[:, lo:hi], k1[:, lo:hi], s2, v2[:, lo:hi],
                    OP.mult, OP.add))
                halves(lambda lo, hi: nc.vector._custom_dve(
                    TURNS_WRAP, out=w2[:, lo:hi], in0=k1[:, lo:hi],
                    in1=v2[:, lo:hi], s0=s2, s1=1.0, imm2=MAGIC))
                halves(lambda lo, hi: nc.scalar.activation(
                    sn2[:, lo:hi], w2[:, lo:hi], AF.Sin, scale=-TWO_PI))
                # q = t + (sg/2) kt1 via a Pool scaled copy (so theta' =
                # q + sg*kt2 and the next wrap starts as soon as kt2 lands)
                nc.gpsimd.tensor_scalar(sk1[:], k1[:], mu, None, OP.mult)
                nc.gpsimd.tensor_tensor(q[:], t_[:], sk1[:], OP.add)
                lp2 = lap_of(p2, "p2")
                halves(lambda lo, hi: nc.vector.scalar_tensor_tensor(
                    k2[:, lo:hi], sn2[:, lo:hi], c_sin, lp2[:, lo:hi],
                    OP.mult, OP.add))

                # ---- eval 3 at p3 = t + s3 kt2, and next step's eval-1
                # wrap+sin (theta' == q + sg*kt2, ready now) ----
                halves(lambda lo, hi: nc.vector.scalar_tensor_tensor(
                    p3[:, lo:hi], k2[:, lo:hi], s3, t_[:, lo:hi],
                    OP.mult, OP.add))
                halves(lambda lo, hi: nc.vector._custom_dve(
                    TURNS_WRAP, out=w3[:, lo:hi], in0=k2[:, lo:hi],
                    in1=t_[:, lo:hi], s0=s3, s1=1.0, imm2=MAGIC))
                halves(lambda lo, hi: nc.scalar.activation(
                    sn3[:, lo:hi], w3[:, lo:hi], AF.Sin, scale=-TWO_PI))
                halves(lambda lo, hi: nc.vector.scalar_tensor_tensor(
                    th_new[:, lo:hi], k2[:, lo:hi], sg, q[:, lo:hi],
                    OP.mult, OP.add))
                lp3 = lap_of(p3, "p3")
                prev_kq[0] = (k2, q)

                # ---- HU' = HU + kp*(kt1 + 4 kt2 + kt3) ----
                nc.gpsimd.tensor_scalar(mm1[:], k1[:], 0.25, None, OP.mult)
                nc.gpsimd.tensor_tensor(m[:], mm1[:], k2[:], OP.add)
                nc.scalar.activation(sm[:], m[:], AF.Copy, bias=0.0,
                                     scale=4.0)
                halves(lambda lo, hi: nc.scalar.activation(
                    cs3[:, lo:hi], sn3[:, lo:hi], AF.Copy, bias=0.0,
                    scale=c_sin))
                halves(lambda lo, hi: nc.gpsimd.tensor_tensor(
                    k3[:, lo:hi], lp3[:, lo:hi], cs3[:, lo:hi], OP.add))
                halves(lambda lo, hi: nc.gpsimd.tensor_tensor(
                    m2[:, lo:hi], sm[:, lo:hi], k3[:, lo:hi], OP.add))
                halves(lambda lo, hi: nc.vector.scalar_tensor_tensor(
                    hu_new[:, lo:hi], m2[:, lo:hi], kp, hu[:, lo:hi],
                    OP.mult, OP.add))

                ths[0], ths[1] = th_new, th
                hus[0], hus[1] = hu_new, hu

            for i in range(nsteps):
                step(i)

            rad = tmp.tile([PB, N], F32, name="rad", tag="rad")
            nc.scalar.activation(rad[:], ths[0][:], AF.Copy, bias=0.0,
                                 scale=TWO_PI)
            nc.gpsimd.dma_start(out[:], rad[:])

    nc.compile()
    return nc


_CACHE: dict = {}


def kernel(x, omega0, coupling, nsteps: int = None):
    x = np.ascontiguousarray(np.asarray(x, dtype=np.float32))
    om = float(np.asarray(omega0, dtype=np.float64))
    cp = float(np.asarray(coupling, dtype=np.float64))
    if nsteps is None:
        nsteps = NSTEPS
    key = (nsteps, om, cp)
    if key not in _CACHE:
        _CACHE[key] = _build(nsteps, om, cp)
    nc = _CACHE[key]

    in_maps = [{"x": x[i * PB:(i + 1) * PB]} for i in range(N_CORES)]
    res = run_bass_kernel_spmd(nc, in_maps, list(range(N_CORES)))
    return np.concatenate(
        [r["out"] for r in res.results], axis=0
    ).astype(np.float32)


# revision 20
# speedup vs baseline: 1.3041x; 1.3041x over previous
"""v6: batch-major Trainium kernel for the coupled-pendulum ODE.

Math (verified on host): classical 3-stage Runge-Kutta-Nystrom order 4,
integrating  theta'' = -omega0^2 sin(theta) + coupling*(ring laplacian)
in rescaled units: tau = omega0*t, angles in TURNS (theta = theta_phys/2pi):
    d2theta/dtau2 = cp*(lap theta) - sin(2pi theta)/(2pi),  cp = coupling/omega0^2.
Working variable kt = theta''/cp = (nsum - 2 theta) + c*ns with
ns = -sin(2pi theta), c = 1/(2pi cp), nsum = theta[n-1] + theta[n+1].

Layout: per core one [128, 512] tile — 128 partitions = batch rows, 512
free = ring positions; the ring stencil is free-dim shifted adds, so no
PE/PSUM and no host transposes.

Engine choice is driven by measured HW chain costs ([128,512] fp32):
DVE scalar_tensor_tensor 246ns (the 2x perf mode is real on HW), DVE
custom 405ns, ACT 570ns, Pool tensor_tensor 1105ns (gpsimd Add runs at
0.42 of roofline), Pool tensor_scalar ~6.7us. So: everything is DVE stt
except the three Sin activations (ACT) and a few early off-critical-path
adds (Pool). The [128,1] ring-edge fixups are DVE stt.

TURNS_WRAP (custom DVE) computes frac(s0*in0+s1*in1) via the 1.5*2^23
magic trick, needed because ACT Sin is only valid on ~[-pi,pi].
"""

import math

import numpy as np

import concourse.bacc as bacc
import concourse.bass as bass
import concourse.dve_ops as dve_ops
import concourse.mybir as mybir
import concourse.tile as tile
from concourse.bass_utils import run_bass_kernel_spmd
from concourse.dve_spec import C0, C1, C2, Spec, Src0, Src1, _has_src1, lower
from concourse.dve_uop import DveOpSpec

F32 = mybir.dt.float32
AF = mybir.ActivationFunctionType
OP = mybir.AluOpType

N_CORES = 8
B, N = 1024, 512
PB = B // N_CORES            # 128 batch rows per core

NSTEPS = 96
T_END = 2.0
TWO_PI = 2 * math.pi
MAGIC = 12582912.0           # 1.5 * 2**23: fp32 round-to-int trick


def _register_custom_op(name, body, reference):
    for op in dve_ops.OPS:
        if op.name == name:
            return op
    idx = dve_ops._CUSTOM_DVE_ROW_BASE + len(dve_ops.OPS)
    assert idx < 0x20
    spec = Spec(body=body, reference=reference)
    shas = {}
    for ver in ("v3", "v4"):
        try:
            uops = lower(spec, ver=ver)
            tmp = DveOpSpec(name=name, opcode=idx, uops=uops,
                            rd1_en=_has_src1(spec))
            shas[ver] = tmp.sha(ver)
        except Exception:
            pass
    op = dve_ops.DveOp(name, spec, subdim=False, uops_sha=shas)
    dve_ops.OPS.append(op)
    dve_ops._SUB_OPCODE_FOR_NAME[name] = idx
    dve_ops.CUSTOM_DVE_SPECS[name] = spec
    return op


def _f32(v):
    return np.float32(v)


_tw_z = Src0 * C0 + Src1 * C1
TURNS_WRAP = _register_custom_op(
    "TURNS_WRAP_ANT",
    _tw_z - ((_tw_z + C2) - C2),
    lambda in0, in1, s0, s1, imm2: (
        lambda z: z - ((z + _f32(imm2)) - _f32(imm2)))(
        (in0.astype(np.float32) * _f32(s0)
         + in1.astype(np.float32) * _f32(s1)).astype(np.float32)),
)


def _build(nsteps: int, omega0: float, coupling: float) -> bass.Bass:
    tau_end = omega0 * T_END
    h = tau_end / nsteps
    cp = coupling / (omega0 * omega0)
    c_sin = 1.0 / (TWO_PI * cp)     # ns coefficient inside kt
    s2 = cp * h * h / 8.0           # p2 = v2 + s2*kt1
    s3 = cp * h * h / 2.0           # p3 = t  + s3*kt2
    sg = cp * h * h / 3.0           # th' = q + sg*kt2
    mu = sg / 2.0                   # q = t + mu*kt1
    kp = cp * h * h / 12.0          # HU' = HU + kp*(kt1 + 4 kt2 + kt3)

    nc = bacc.Bacc("TRN2", target_bir_lowering=False, debug=False,
                   num_devices=N_CORES)
    x_in = nc.dram_tensor("x", [PB, N], F32, kind="ExternalInput")
    out = nc.dram_tensor("out", [PB, N], F32, kind="ExternalOutput")

    with tile.TileContext(nc) as tc:
        with (
            tc.tile_pool(name="state", bufs=1) as state,
            tc.tile_pool(name="tmp", bufs=2) as tmp,
        ):
            th = state.tile([PB, N], F32, name="th", tag="th")
            th2 = state.tile([PB, N], F32, name="th2", tag="th2")
            hu = state.tile([PB, N], F32, name="hu", tag="hu")
            hu2 = state.tile([PB, N], F32, name="hu2", tag="hu2")

            xs = tmp.tile([PB, N], F32, name="xs", tag="xs")
            nc.gpsimd.dma_start(xs[:], x_in[:])
            # theta0 = x - 0.5 (turns); HU = (h/2) u = 0
            nc.scalar.activation(th[:], xs[:], AF.Copy, bias=-0.5, scale=1.0)
            nc.vector.memset(hu[:], 0.0)

            ths = [th, th2]
            hus = [hu, hu2]
            # (k2, q) of the previous step: theta' == q + sg*k2, so the next
            # wrap starts without waiting for theta' to materialize.
            prev_kq = [None]
            # (v2, t) = (theta+HU, theta+2HU) for the CURRENT step, computed
            # near the end of the previous step, off the critical path.
            vt = [None]

            def stt(o, a, s, b):
                nc.vector.scalar_tensor_tensor(o, a, s, b, OP.mult, OP.add)

            def nsum_of(p, nm):
                """ns[n] = p[n-1] + p[n+1] with ring wraparound: one main
                DVE stt + two [128,1] edge fixups."""
                ns_t = tmp.tile([PB, N], F32, name=nm, tag=nm)
                stt(ns_t[:, 1:N - 1], p[:, 0:N - 2], 1.0, p[:, 2:N])
                stt(ns_t[:, 0:1], p[:, N - 1:N], 1.0, p[:, 1:2])
                stt(ns_t[:, N - 1:N], p[:, N - 2:N - 1], 1.0, p[:, 0:1])
                return ns_t

            def step(i):
                th, th_new = ths
                hu, hu_new = hus
                q = tmp.tile([PB, N], F32, name="q", tag="q")
                w1 = tmp.tile([PB, N], F32, name="w1", tag="w1")
                w2 = tmp.tile([PB, N], F32, name="w2", tag="w2")
                w3 = tmp.tile([PB, N], F32, name="w3", tag="w3")
                sn1 = tmp.tile([PB, N], F32, name="sn1", tag="sn1")
                sn2 = tmp.tile([PB, N], F32, name="sn2", tag="sn2")
                sn3 = tmp.tile([PB, N], F32, name="sn3", tag="sn3")
                na1 = tmp.tile([PB, N], F32, name="na1", tag="na1")
                na2 = tmp.tile([PB, N], F32, name="na2", tag="na2")
                na3 = tmp.tile([PB, N], F32, name="na3", tag="na3")
                k1 = tmp.tile([PB, N], F32, name="k1", tag="k1")
                k2 = tmp.tile([PB, N], F32, name="k2", tag="k2")
                k3 = tmp.tile([PB, N], F32, name="k3", tag="k3")
                p2 = tmp.tile([PB, N], F32, name="p2", tag="p2")
                p3 = tmp.tile([PB, N], F32, name="p3", tag="p3")
                m = tmp.tile([PB, N], F32, name="m", tag="m")
                m2 = tmp.tile([PB, N], F32, name="m2", tag="m2")

                if vt[0] is None:
                    v2 = tmp.tile([PB, N], F32, name="v2", tag="v2")
                    t_ = tmp.tile([PB, N], F32, name="t_", tag="t_")
                    nc.gpsimd.tensor_tensor(v2[:], th[:], hu[:], OP.add)
                    nc.gpsimd.tensor_tensor(t_[:], v2[:], hu[:], OP.add)
                else:
                    v2, t_ = vt[0]

                # ---- eval 1 at theta ----
                if prev_kq[0] is None:
                    nc.vector._custom_dve(TURNS_WRAP, out=w1[:], in0=th[:],
                                          in1=th[:], s0=1.0, s1=0.0,
                                          imm2=MAGIC)
                else:
                    pk, pq = prev_kq[0]
                    nc.vector._custom_dve(TURNS_WRAP, out=w1[:], in0=pk[:],
                                          in1=pq[:], s0=sg, s1=1.0,
                                          imm2=MAGIC)
                nc.scalar.activation(sn1[:], w1[:], AF.Sin, scale=-TWO_PI)
                ns1 = nsum_of(th, "ns1")
                stt(na1[:], sn1[:], c_sin, ns1[:])
                stt(k1[:], th[:], -2.0, na1[:])

                # ---- eval 2 at p2 = v2 + s2 kt1 ----
                stt(p2[:], k1[:], s2, v2[:])
                nc.vector._custom_dve(TURNS_WRAP, out=w2[:], in0=k1[:],
                                      in1=v2[:], s0=s2, s1=1.0, imm2=MAGIC)
                nc.scalar.activation(sn2[:], w2[:], AF.Sin, scale=-TWO_PI)
                stt(q[:], k1[:], mu, t_[:])
                ns2 = nsum_of(p2, "ns2")
                stt(na2[:], sn2[:], c_sin, ns2[:])
                stt(k2[:], p2[:], -2.0, na2[:])

                # ---- eval 3 at p3 = t + s3 kt2 ----
                stt(p3[:], k2[:], s3, t_[:])
                nc.vector._custom_dve(TURNS_WRAP, out=w3[:], in0=k2[:],
                                      in1=t_[:], s0=s3, s1=1.0, imm2=MAGIC)
                nc.scalar.activation(sn3[:], w3[:], AF.Sin, scale=-TWO_PI)
                # theta' = q + sg*kt2 (needed by eval 1 of the next step)
                stt(th_new[:], k2[:], sg, q[:])
                stt(m[:], k1[:], 0.25, k2[:])
                ns3 = nsum_of(p3, "ns3")
                stt(na3[:], sn3[:], c_sin, ns3[:])
                stt(k3[:], p3[:], -2.0, na3[:])

                # ---- HU' = HU + kp*(kt1 + 4 kt2 + kt3) ----
                stt(m2[:], m[:], 4.0, k3[:])
                stt(hu_new[:], m2[:], kp, hu[:])
                # v2' = theta' + HU' via (theta'+HU) + kp*m2 so the next
                # step's position partial doesn't wait on a Pool op; the
                # Pool adds (va, t') run early / off the critical path.
                va = tmp.tile([PB, N], F32, name="va", tag="va")
                vn = tmp.tile([PB, N], F32, name="vn", tag="vn")
                tn = tmp.tile([PB, N], F32, name="tn", tag="tn")
                nc.gpsimd.tensor_tensor(va[:], th_new[:], hu[:], OP.add)
                stt(vn[:], m2[:], kp, va[:])
                nc.gpsimd.tensor_tensor(tn[:], vn[:], hu_new[:], OP.add)

                prev_kq[0] = (k2, q)
                vt[0] = (vn, tn)
                ths[0], ths[1] = th_new, th
                hus[0], hus[1] = hu_new, hu

            for i in range(nsteps):
                step(i)

            rad = tmp.tile([PB, N], F32, name="rad", tag="rad")
            nc.scalar.activation(rad[:], ths[0][:], AF.Copy, bias=0.0,
                                 scale=TWO_PI)
            nc.gpsimd.dma_start(out[:], rad[:])

    nc.compile()
    return nc


_CACHE: dict = {}


def kernel(x, omega0, coupling, nsteps: int = None):
    x = np.ascontiguousarray(np.asarray(x, dtype=np.float32))
    om = float(np.asarray(omega0, dtype=np.float64))
    cp = float(np.asarray(coupling, dtype=np.float64))
    if nsteps is None:
        nsteps = NSTEPS
    key = (nsteps, om, cp)
    if key not in _CACHE:
        _CACHE[key] = _build(nsteps, om, cp)
    nc = _CACHE[key]

    in_maps = [{"x": x[i * PB:(i + 1) * PB]} for i in range(N_CORES)]
    res = run_bass_kernel_spmd(nc, in_maps, list(range(N_CORES)))
    return np.concatenate(
        [r["out"] for r in res.results], axis=0
    ).astype(np.float32)


# revision 23
# speedup vs baseline: 2.0382x; 1.5629x over previous
"""v6: batch-major Trainium kernel for the coupled-pendulum ODE.

Math (verified on host): classical 3-stage Runge-Kutta-Nystrom order 4,
integrating  theta'' = -omega0^2 sin(theta) + coupling*(ring laplacian)
in rescaled units: tau = omega0*t, angles in TURNS (theta = theta_phys/2pi):
    d2theta/dtau2 = cp*(lap theta) - sin(2pi theta)/(2pi),  cp = coupling/omega0^2.
Working variable kt = theta''/cp = (nsum - 2 theta) + c*ns with
ns = -sin(2pi theta), c = 1/(2pi cp), nsum = theta[n-1] + theta[n+1].

Layout: per core one [128, 512] tile — 128 partitions = batch rows, 512
free = ring positions; the ring stencil is free-dim shifted adds, so no
PE/PSUM and no host transposes.

Engine choice is driven by measured HW chain costs ([128,512] fp32):
DVE scalar_tensor_tensor 246ns (the 2x perf mode is real on HW), DVE
custom 405ns, ACT 570ns, Pool tensor_tensor 1105ns (gpsimd Add runs at
0.42 of roofline), Pool tensor_scalar ~6.7us. So: everything is DVE stt
except the three Sin activations (ACT) and a few early off-critical-path
adds (Pool). The [128,1] ring-edge fixups are DVE stt.

TURNS_WRAP (custom DVE) computes frac(s0*in0+s1*in1) via the 1.5*2^23
magic trick, needed because ACT Sin is only valid on ~[-pi,pi].
"""

import math

import numpy as np

import concourse.bacc as bacc
import concourse.bass as bass
import concourse.dve_ops as dve_ops
import concourse.mybir as mybir
import concourse.tile as tile
from concourse.bass_utils import run_bass_kernel_spmd
from concourse.dve_spec import C0, C1, C2, Spec, Src0, Src1, _has_src1, lower
from concourse.dve_uop import DveOpSpec

F32 = mybir.dt.float32
AF = mybir.ActivationFunctionType
OP = mybir.AluOpType

N_CORES = 8
B, N = 1024, 512
PB = B // N_CORES            # 128 batch rows per core

NSTEPS = 80
T_END = 2.0
TWO_PI = 2 * math.pi
MAGIC = 12582912.0           # 1.5 * 2**23: fp32 round-to-int trick


def _register_custom_op(name, body, reference):
    for op in dve_ops.OPS:
        if op.name == name:
            return op
    idx = dve_ops._CUSTOM_DVE_ROW_BASE + len(dve_ops.OPS)
    assert idx < 0x20
    spec = Spec(body=body, reference=reference)
    shas = {}
    for ver in ("v3", "v4"):
        try:
            uops = lower(spec, ver=ver)
            tmp = DveOpSpec(name=name, opcode=idx, uops=uops,
                            rd1_en=_has_src1(spec))
            shas[ver] = tmp.sha(ver)
        except Exception:
            pass
    op = dve_ops.DveOp(name, spec, subdim=False, uops_sha=shas)
    dve_ops.OPS.append(op)
    dve_ops._SUB_OPCODE_FOR_NAME[name] = idx
    dve_ops.CUSTOM_DVE_SPECS[name] = spec
    return op


def _f32(v):
    return np.float32(v)


_tw_z = Src0 * C0 + Src1 * C1
TURNS_WRAP = _register_custom_op(
    "TURNS_WRAP_ANT",
    _tw_z - ((_tw_z + C2) - C2),
    lambda in0, in1, s0, s1, imm2: (
        lambda z: z - ((z + _f32(imm2)) - _f32(imm2)))(
        (in0.astype(np.float32) * _f32(s0)
         + in1.astype(np.float32) * _f32(s1)).astype(np.float32)),
)


def _build(nsteps: int, omega0: float, coupling: float) -> bass.Bass:
    tau_end = omega0 * T_END
    h = tau_end / nsteps
    cp = coupling / (omega0 * omega0)
    c_sin = 1.0 / (TWO_PI * cp)     # ns coefficient inside kt
    s2 = cp * h * h / 8.0           # p2 = v2 + s2*kt1
    s3 = cp * h * h / 2.0           # p3 = t  + s3*kt2
    sg = cp * h * h / 3.0           # th' = q + sg*kt2
    mu = sg / 2.0                   # q = t + mu*kt1
    kp = cp * h * h / 12.0          # HU' = HU + kp*(kt1 + 4 kt2 + kt3)

    nc = bacc.Bacc("TRN2", target_bir_lowering=False, debug=False,
                   num_devices=N_CORES)
    x_in = nc.dram_tensor("x", [PB, N], F32, kind="ExternalInput")
    out = nc.dram_tensor("out", [PB, N], F32, kind="ExternalOutput")

    with tile.TileContext(nc) as tc:
        with (
            tc.tile_pool(name="state", bufs=1) as state,
            tc.tile_pool(name="tmp", bufs=2) as tmp,
        ):
            th = state.tile([PB, N], F32, name="th", tag="th")
            th2 = state.tile([PB, N], F32, name="th2", tag="th2")
            hu = state.tile([PB, N], F32, name="hu", tag="hu")
            hu2 = state.tile([PB, N], F32, name="hu2", tag="hu2")

            xs = tmp.tile([PB, N], F32, name="xs", tag="xs")
            nc.gpsimd.dma_start(xs[:], x_in[:])
            # theta0 = x - 0.5 (turns); HU = (h/2) u = 0
            nc.scalar.activation(th[:], xs[:], AF.Copy, bias=-0.5, scale=1.0)
            nc.vector.memset(hu[:], 0.0)

            ths = [th, th2]
            hus = [hu, hu2]
            # (k2, q) of the previous step: theta' == q + sg*k2, so the next
            # wrap starts without waiting for theta' to materialize.
            prev_kq = [None]
            # (v2, t) = (theta+HU, theta+2HU) for the CURRENT step, computed
            # near the end of the previous step, off the critical path.
            vt = [None]

            def stt(o, a, s, b):
                nc.vector.scalar_tensor_tensor(o, a, s, b, OP.mult, OP.add)

            def nsum_of(p, nm, pool=False):
                """ns[n] = p[n-1] + p[n+1] with ring wraparound: one main
                op (DVE stt, or Pool tensor_tensor for the off-critical
                stencils) + two [128,1] DVE edge fixups ([128,1] on gpsimd
                crashes the exec unit, so the tinies stay on DVE)."""
                ns_t = tmp.tile([PB, N], F32, name=nm, tag=nm)
                if pool:
                    nc.gpsimd.tensor_tensor(ns_t[:, 1:N - 1], p[:, 0:N - 2],
                                            p[:, 2:N], OP.add)
                else:
                    stt(ns_t[:, 1:N - 1], p[:, 0:N - 2], 1.0, p[:, 2:N])
                stt(ns_t[:, 0:1], p[:, N - 1:N], 1.0, p[:, 1:2])
                stt(ns_t[:, N - 1:N], p[:, N - 2:N - 1], 1.0, p[:, 0:1])
                return ns_t

            def step(i):
                th, th_new = ths
                hu, hu_new = hus
                q = tmp.tile([PB, N], F32, name="q", tag="q")
                w1 = tmp.tile([PB, N], F32, name="w1", tag="w1")
                w2 = tmp.tile([PB, N], F32, name="w2", tag="w2")
                w3 = tmp.tile([PB, N], F32, name="w3", tag="w3")
                sn1 = tmp.tile([PB, N], F32, name="sn1", tag="sn1")
                sn2 = tmp.tile([PB, N], F32, name="sn2", tag="sn2")
                sn3 = tmp.tile([PB, N], F32, name="sn3", tag="sn3")
                na1 = tmp.tile([PB, N], F32, name="na1", tag="na1")
                na2 = tmp.tile([PB, N], F32, name="na2", tag="na2")
                na3 = tmp.tile([PB, N], F32, name="na3", tag="na3")
                k1 = tmp.tile([PB, N], F32, name="k1", tag="k1")
                k2 = tmp.tile([PB, N], F32, name="k2", tag="k2")
                k3 = tmp.tile([PB, N], F32, name="k3", tag="k3")
                p2 = tmp.tile([PB, N], F32, name="p2", tag="p2")
                p3 = tmp.tile([PB, N], F32, name="p3", tag="p3")
                m = tmp.tile([PB, N], F32, name="m", tag="m")
                m2 = tmp.tile([PB, N], F32, name="m2", tag="m2")

                if vt[0] is None:
                    v2 = tmp.tile([PB, N], F32, name="v2", tag="v2")
                    t_ = tmp.tile([PB, N], F32, name="t_", tag="t_")
                    nc.gpsimd.tensor_tensor(v2[:], th[:], hu[:], OP.add)
                    nc.gpsimd.tensor_tensor(t_[:], v2[:], hu[:], OP.add)
                else:
                    v2, t_ = vt[0]

                # ---- eval 1 at theta ----
                if prev_kq[0] is None:
                    nc.vector._custom_dve(TURNS_WRAP, out=w1[:], in0=th[:],
                                          in1=th[:], s0=1.0, s1=0.0,
                                          imm2=MAGIC)
                else:
                    pk, pq = prev_kq[0]
                    nc.vector._custom_dve(TURNS_WRAP, out=w1[:], in0=pk[:],
                                          in1=pq[:], s0=sg, s1=1.0,
                                          imm2=MAGIC)
                nc.scalar.activation(sn1[:], w1[:], AF.Sin, scale=-TWO_PI)
                ns1 = nsum_of(th, "ns1", pool=True)
                stt(na1[:], sn1[:], c_sin, ns1[:])
                stt(k1[:], th[:], -2.0, na1[:])

                # ---- eval 2 at p2 = v2 + s2 kt1 ----
                stt(p2[:], k1[:], s2, v2[:])
                nc.vector._custom_dve(TURNS_WRAP, out=w2[:], in0=k1[:],
                                      in1=v2[:], s0=s2, s1=1.0, imm2=MAGIC)
                nc.scalar.activation(sn2[:], w2[:], AF.Sin, scale=-TWO_PI)
                stt(q[:], k1[:], mu, t_[:])
                ns2 = nsum_of(p2, "ns2")
                stt(na2[:], sn2[:], c_sin, ns2[:])
                stt(k2[:], p2[:], -2.0, na2[:])

                # ---- eval 3 at p3 = t + s3 kt2 ----
                stt(p3[:], k2[:], s3, t_[:])
                nc.vector._custom_dve(TURNS_WRAP, out=w3[:], in0=k2[:],
                                      in1=t_[:], s0=s3, s1=1.0, imm2=MAGIC)
                nc.scalar.activation(sn3[:], w3[:], AF.Sin, scale=-TWO_PI)
                # theta' = q + sg*kt2 (needed by eval 1 of the next step)
                stt(th_new[:], k2[:], sg, q[:])
                stt(m[:], k1[:], 0.25, k2[:])
                ns3 = nsum_of(p3, "ns3", pool=True)
                stt(na3[:], sn3[:], c_sin, ns3[:])
                stt(k3[:], p3[:], -2.0, na3[:])

                # ---- HU' = HU + kp*(kt1 + 4 kt2 + kt3) ----
                stt(m2[:], m[:], 4.0, k3[:])
                stt(hu_new[:], m2[:], kp, hu[:])
                # v2' = theta' + HU' via (theta'+HU) + kp*m2 so the next
                # step's position partial doesn't wait on a Pool op; the
                # Pool adds (va, t') run early / off the critical path.
                va = tmp.tile([PB, N], F32, name="va", tag="va")
                vn = tmp.tile([PB, N], F32, name="vn", tag="vn")
                tn = tmp.tile([PB, N], F32, name="tn", tag="tn")
                nc.gpsimd.tensor_tensor(va[:], th_new[:], hu[:], OP.add)
                stt(vn[:], m2[:], kp, va[:])
                nc.gpsimd.tensor_tensor(tn[:], vn[:], hu_new[:], OP.add)

                prev_kq[0] = (k2, q)
                vt[0] = (vn, tn)
                ths[0], ths[1] = th_new, th
                hus[0], hus[1] = hu_new, hu

            for i in range(nsteps):
                step(i)

            rad = tmp.tile([PB, N], F32, name="rad", tag="rad")
            nc.scalar.activation(rad[:], ths[0][:], AF.Copy, bias=0.0,
                                 scale=TWO_PI)
            nc.gpsimd.dma_start(out[:], rad[:])

    nc.compile()
    return nc


_CACHE: dict = {}


def kernel(x, omega0, coupling, nsteps: int = None):
    x = np.ascontiguousarray(np.asarray(x, dtype=np.float32))
    om = float(np.asarray(omega0, dtype=np.float64))
    cp = float(np.asarray(coupling, dtype=np.float64))
    if nsteps is None:
        nsteps = NSTEPS
    key = (nsteps, om, cp)
    if key not in _CACHE:
        _CACHE[key] = _build(nsteps, om, cp)
    nc = _CACHE[key]

    in_maps = [{"x": x[i * PB:(i + 1) * PB]} for i in range(N_CORES)]
    res = run_bass_kernel_spmd(nc, in_maps, list(range(N_CORES)))
    return np.concatenate(
        [r["out"] for r in res.results], axis=0
    ).astype(np.float32)
